# revision 1
# baseline (speedup 1.0000x reference)
"""LocalWindowAttention (3x3 windows, B=16, 96x96, C=256, 4 heads) on 8
Trainium2 NeuronCores via Bass/Tile. Pure data parallel: 2 images per core.

Self-contained: builds the per-core Bass program, shards the batch, runs
SPMD on cores 0-7, gathers the full output.
"""

import numpy as np
import ml_dtypes

import concourse.bass as bass
import concourse.bacc as bacc
import concourse.tile as tile
from concourse import mybir
from concourse.bass_utils import run_bass_kernel_spmd

F32 = mybir.dt.float32
BF16 = mybir.dt.bfloat16

B = 16
NCORES = 8
IMG = B // NCORES          # images per core
C = 256
NH = 4
HD = 64
WS = 3
GRID = 96                  # H = W = 96
S = GRID // WS             # 32 window-rows ("strips") per image
NT = GRID * GRID           # tokens per image
SCALE = HD ** -0.5
BLOCKS = [(0, 14), (14, 14), (28, 4)]  # (wcol0, nwin); block tokens = 9*nwin


def _build(nc, ns=S, img=IMG, reps=1):
    nt = ns * 288
    x = nc.declare_dram_parameter("x", [img, nt, C], F32, isOutput=False).ap()
    wqkvT = nc.declare_dram_parameter("wqkvT", [128, 2, 768], BF16, isOutput=False).ap()
    wprojT = nc.declare_dram_parameter("wprojT", [128, 2, 256], BF16, isOutput=False).ap()
    maskc = nc.declare_dram_parameter("maskc", [128, 256], BF16, isOutput=False).ap()
    identc = nc.declare_dram_parameter("identc", [128, 128], BF16, isOutput=False).ap()
    onesc = nc.declare_dram_parameter("onesc", [128, 128], BF16, isOutput=False).ap()
    y = nc.declare_dram_parameter("y", [img, nt, C], F32, isOutput=True).ap()

    # [img, wrow, r, col, chan]; block b covers cols 42b : 42b+3*nwin
    xv = x.rearrange("b (wr r col) ch -> b wr r col ch", r=WS, col=GRID)
    yv = y.rearrange("b (wr r col) ch -> b wr r col ch", r=WS, col=GRID)

    with tile.TileContext(nc) as tc:
        with (
            tc.tile_pool(name="const", bufs=1) as constp,
            tc.tile_pool(name="sb", bufs=1) as sb,
            tc.tile_pool(name="ps", bufs=1, space="PSUM") as ps,
        ):
            wq_sb = constp.tile([128, 2, 768], BF16)
            nc.sync.dma_start(out=wq_sb[:], in_=wqkvT[:])
            wp_sb = constp.tile([128, 2, 256], BF16)
            nc.sync.dma_start(out=wp_sb[:], in_=wprojT[:])
            mask_sb = constp.tile([128, 256], BF16)
            nc.sync.dma_start(out=mask_sb[:], in_=maskc[:])
            ident_sb = constp.tile([128, 128], BF16)
            nc.sync.dma_start(out=ident_sb[:], in_=identc[:])
            ones_sb = constp.tile([128, 128], BF16)
            nc.sync.dma_start(out=ones_sb[:], in_=onesc[:])

            def _all():
                for b_ in range(img):
                    for w in range(ns):
                        _strip(nc, sb, ps, xv, yv, b_, w,
                               wq_sb, wp_sb, mask_sb, ident_sb, ones_sb)

            if reps == 1:
                _all()
            else:
                with tc.For_i(0, reps, 1):
                    _all()
    return nc


def _strip(nc, sb, ps, xv, yv, img, w, wq_sb, wp_sb, mask_sb, ident_sb, ones_sb):
    # ---- load x (strip = one window-row = 288 tokens), per-block DMAs ----
    x_sb = sb.tile([128, 3, C], F32, tag="x", bufs=3)
    for b, (wc0, nb) in enumerate(BLOCKS):
        nc.sync.dma_start(
            out=x_sb[0 : nb * 9, b, :],
            in_=xv[img, w, :, 3 * wc0 : 3 * (wc0 + nb), :],
        )

    # ---- cast to bf16 (gpsimd) ----
    x_bf = sb.tile([128, 3, C], BF16, tag="xbf", bufs=2)
    for b, (wc0, nb) in enumerate(BLOCKS):
        kb = nb * 9
        nc.gpsimd.tensor_copy(out=x_bf[0:kb, b, :], in_=x_sb[0:kb, b, :])

    # ---- x^T via PE transpose: [128ch, 288tok] in 2 chan-chunks ----
    xT_ps = ps.tile([128, 2, 288], BF16, tag="xT", bufs=1)
    for b, (wc0, nb) in enumerate(BLOCKS):
        kb = nb * 9
        for cc in range(2):
            nc.tensor.transpose(
                out=xT_ps[:, cc, 126 * b : 126 * b + kb],
                in_=x_bf[0:kb, b, 128 * cc : 128 * cc + 128],
                identity=ident_sb[0:kb, 0:kb],
            )
    xT_sb = sb.tile([128, 2, 288], BF16, tag="xTs", bufs=2)
    nc.vector.tensor_copy(out=xT_sb[:], in_=xT_ps[:])

    # ---- q^T, k^T channel-major; chunk mc holds heads (2mc, 2mc+1) ----
    qkT_ps = []
    for t, base in ((0, 0), (1, 256)):  # t=0 -> q, t=1 -> k
        chunks = []
        for mc in range(2):
            qp = ps.tile([128, 288], F32, tag="qk", bufs=2)
            for kc in range(2):
                nc.tensor.matmul(
                    out=qp[:],
                    lhsT=wq_sb[:, kc, base + 128 * mc : base + 128 * mc + 128],
                    rhs=xT_sb[:, kc, :],
                    start=(kc == 0),
                    stop=(kc == 1),
                )
            chunks.append(qp)
        qkT_ps.append(chunks)
    qT_sb = sb.tile([128, 2, 288], BF16, tag="qTs", bufs=2)
    kT_sb = sb.tile([128, 2, 288], BF16, tag="kTs", bufs=2)
    for mc in range(2):
        nc.vector.tensor_copy(out=qT_sb[:, mc, :], in_=qkT_ps[0][mc][:])
        nc.scalar.copy(out=kT_sb[:, mc, :], in_=qkT_ps[1][mc][:])

    # ---- v token-major: per block [kb, 256] ----
    v_sb = sb.tile([128, 3, C], BF16, tag="vs", bufs=2)
    for b, (wc0, nb) in enumerate(BLOCKS):
        kb = nb * 9
        vp = ps.tile([128, 512], F32, tag="sm", bufs=3)
        for kc in range(2):
            nc.tensor.matmul(
                out=vp[0:kb, 0:256],
                lhsT=xT_sb[:, kc, 126 * b : 126 * b + kb],
                rhs=wq_sb[:, kc, 512:768],
                start=(kc == 0),
                stop=(kc == 1),
            )
        nc.scalar.copy(out=v_sb[0:kb, b, :], in_=vp[0:kb, 0:256])

    # ---- QK^T logits^T per block/head: [k, q] ----
    # expm head order within a block: [h0, h2, h1, h3] — even-row-group MMs
    # land in bank E, odd-row-group in bank O (same-bank mixed row groups
    # are an unrecoverable HW fault).
    expm = sb.tile([126, 12, 126], BF16, tag="expm", bufs=2)
    for b, (wc0, nb) in enumerate(BLOCKS):
        kb = nb * 9
        # row-group hh writes its own PSUM bank (free-offset 512*hh):
        # mixing row groups within one bank is an unrecoverable HW fault.
        aL = ps.tile([128, 2, 512], F32, tag="att", bufs=1)
        for mc in range(2):
            for hh in range(2):
                p0 = 64 * hh
                nc.tensor.matmul(
                    out=aL[0:kb, hh, 126 * mc : 126 * mc + kb],
                    lhsT=kT_sb[p0 : p0 + 64, mc, 126 * b : 126 * b + kb],
                    rhs=qT_sb[p0 : p0 + 64, mc, 126 * b : 126 * b + kb],
                    start=True,
                    stop=True,
                )
        # one exp op over both banks; head order per block: h0, h2, h1, h3
        ein = bass.AP(tensor=aL.tensor, offset=aL.offset,
                      ap=[[aL.ap[0][0], kb], [512, 2], [126, 2], [1, kb]])
        nc.scalar.activation(
            out=expm[0:kb, 4 * b : 4 * b + 4, 0:kb], in_=ein,
            func=mybir.ActivationFunctionType.Exp, scale=SCALE)

    # ---- mask (DVE): expm *= blockdiag(9) ----
    m = mask_sb[0:126, 0:126]
    mb = bass.AP(tensor=m.tensor, offset=m.offset,
                 ap=[m.ap[0], [0, 8], m.ap[1]])
    nc.vector.tensor_mul(
        out=expm[:, 0:8, :], in0=expm[:, 0:8, :], in1=mb)
    m2 = mask_sb[0:36, 128 : 128 + 36]
    mb2 = bass.AP(tensor=m2.tensor, offset=m2.offset,
                  ap=[m2.ap[0], [0, 4], m2.ap[1]])
    nc.vector.tensor_mul(
        out=expm[0:36, 8:12, 0:36], in0=expm[0:36, 8:12, 0:36], in1=mb2)

    # ---- denominators broadcast over 64-row groups via ones-matmul ----
    rbc = []
    for T in range(2):
        dp = ps.tile([128, 512], F32, tag="sm", bufs=3)
        for hh in range(2):
            h = 2 * T + hh
            hc = (h % 2) * 2 + h // 2
            e01 = expm[:, hc, :]
            e01 = bass.AP(tensor=e01.tensor, offset=e01.offset,
                          ap=[e01.ap[0], [504, 2], [1, 126]])
            nc.tensor.matmul(
                out=dp[64 * hh : 64 * hh + 64, 0:252],
                lhsT=ones_sb[0:126, 0:64],
                rhs=e01,
                start=True, stop=True,
                tile_position=(0, 64 * hh),
            )
            nc.tensor.matmul(
                out=dp[64 * hh : 64 * hh + 64, 252:288],
                lhsT=ones_sb[0:36, 0:64],
                rhs=expm[0:36, 8 + hc, 0:36],
                start=True, stop=True,
                tile_position=(0, 64 * hh),
            )
        r = sb.tile([128, 288], F32, tag="rbc", bufs=2)
        nc.vector.reciprocal_approx_fast(out=r[:, 0:288], in_=dp[:, 0:288])
        rbc.append(r)

    # ---- AV: unnormalized channel-major ao; normalize during evac ----
    ao_sb = sb.tile([128, 2, 288], BF16, tag="aos", bufs=2)
    for T in range(2):
        ap_ = ps.tile([128, 512], F32, tag="sm", bufs=3)
        for hh in range(2):
            h = 2 * T + hh
            hc = (h % 2) * 2 + h // 2
            for b, (wc0, nb) in enumerate(BLOCKS):
                kb = nb * 9
                nc.tensor.matmul(
                    out=ap_[64 * hh : 64 * hh + 64, 126 * b : 126 * b + kb],
                    lhsT=v_sb[0:kb, b, 64 * h : 64 * h + 64],
                    rhs=expm[0:kb, 4 * b + hc, 0:kb],
                    start=True, stop=True,
                    tile_position=(0, 64 * hh),
                )
        nc.vector.tensor_mul(out=ao_sb[:, T, 0:288], in0=ap_[:, 0:288],
                             in1=rbc[T][:, 0:288])

    # ---- proj + output evac + scatter ----
    out_sb = sb.tile([128, 3, C], F32, tag="outs", bufs=2)
    for b, (wc0, nb) in enumerate(BLOCKS):
        kb = nb * 9
        op = ps.tile([128, 512], F32, tag="sm", bufs=3)
        for T in range(2):
            nc.tensor.matmul(
                out=op[0:kb, 0:256],
                lhsT=ao_sb[:, T, 126 * b : 126 * b + kb],
                rhs=wp_sb[:, T, :],
                start=(T == 0),
                stop=(T == 1),
            )
        nc.scalar.copy(out=out_sb[0:kb, b, :], in_=op[0:kb, 0:256])
        nc.sync.dma_start(
            out=yv[img, w, :, 3 * wc0 : 3 * (wc0 + nb), :],
            in_=out_sb[0 : nb * 9, b, :],
        )


def _make_consts():
    bf16 = ml_dtypes.bfloat16
    mask = np.zeros((128, 256), np.float32)
    for p in range(126):
        for q in range(126):
            if (p % 42) // 3 == (q % 42) // 3:
                mask[p, q] = 1.0
    for p in range(36):
        for q in range(36):
            if (p % 12) // 3 == (q % 12) // 3:
                mask[p, 128 + q] = 1.0
    return {
        "maskc": mask.astype(bf16),
        "identc": np.eye(128, dtype=np.float32).astype(bf16),
        "onesc": np.ones((128, 128), np.float32).astype(bf16),
    }


_NC_CACHE = {}


def _get_nc():
    if "nc" not in _NC_CACHE:
        nc = bacc.Bacc("TRN2", target_bir_lowering=False, debug=False,
                       num_devices=NCORES)
        _build(nc)
        nc.compile()
        _NC_CACHE["nc"] = nc
    return _NC_CACHE["nc"]


def _in_maps(x, Wqkv, Wproj):
    bf16 = ml_dtypes.bfloat16
    consts = _make_consts()
    consts["wqkvT"] = np.ascontiguousarray(
        np.asarray(Wqkv, np.float32).T.reshape(2, 128, 768).transpose(1, 0, 2)
    ).astype(bf16)
    consts["wprojT"] = np.ascontiguousarray(
        np.asarray(Wproj, np.float32).T.reshape(2, 128, 256).transpose(1, 0, 2)
    ).astype(bf16)
    x = np.asarray(x, np.float32)
    return [{"x": x[IMG * c : IMG * c + IMG], **consts} for c in range(NCORES)]


def kernel(x, Wqkv, Wproj, H, W):
    assert int(H) == GRID and int(W) == GRID
    nc = _get_nc()
    res = run_bass_kernel_spmd(nc, _in_maps(x, Wqkv, Wproj), list(range(NCORES)))
    out = np.concatenate([res.results[c]["y"] for c in range(NCORES)], axis=0)
    return np.ascontiguousarray(out.reshape(B, NT, C))



# revision 3
# speedup vs baseline: 4.3978x; 4.3978x over previous
"""LocalWindowAttention (3x3 windows, B=16, 96x96, C=256, 4 heads) on 8
Trainium2 NeuronCores via Bass/Tile. Pure data parallel: 2 images per core.

v2: host pre-permutes x to bf16 window-contiguous (col-major within strips)
split by channel half, so each 7-strip round loads with 2 xbar transpose-DMAs
straight into channel-major SBUF. Attention runs on uniform 14-window
(126-token) tiles; output is written bf16 in one DMA per round and
un-permuted on the host.
"""

import numpy as np
import ml_dtypes

import concourse.bass as bass
import concourse.bacc as bacc
import concourse.tile as tile
from concourse import mybir
from concourse.bass_utils import run_bass_kernel_spmd

F32 = mybir.dt.float32
BF16 = mybir.dt.bfloat16

B = 16
NCORES = 8
IMG = B // NCORES          # images per core
C = 256
NH = 4
HD = 64
WS = 3
GRID = 96
NSTRIP = 32                # window-rows per image
NT = GRID * GRID           # tokens per image
SCALE = HD ** -0.5

# rounds per image: (start_strip, n_strips). 7-strip rounds hold exactly 16
# 14-window tiles; the 4-strip tail holds 9 full tiles + one 2-window tile.
ROUNDS = [(0, 7), (7, 7), (14, 7), (21, 7), (28, 4)]


def _tiles_for(nstrips):
    nw = nstrips * 32          # windows in round
    full, rem = divmod(nw, 14)
    t = [14] * full
    if rem:
        t.append(rem)
    return t


def _build(nc, img=IMG, reps=1):
    x = nc.declare_dram_parameter("x", [img, 2, NT, 128], BF16, isOutput=False).ap()
    wqkvT = nc.declare_dram_parameter("wqkvT", [128, 2, 768], BF16, isOutput=False).ap()
    wprojT = nc.declare_dram_parameter("wprojT", [128, 2, 256], BF16, isOutput=False).ap()
    maskc = nc.declare_dram_parameter("maskc", [128, 128], BF16, isOutput=False).ap()
    onesc = nc.declare_dram_parameter("onesc", [128, 64], BF16, isOutput=False).ap()
    y = nc.declare_dram_parameter("y", [img, NT, C], BF16, isOutput=True).ap()

    with tile.TileContext(nc) as tc:
        with (
            tc.tile_pool(name="const", bufs=1) as constp,
            tc.tile_pool(name="sb", bufs=1) as sb,
            tc.tile_pool(name="ps", bufs=1, space="PSUM") as ps,
        ):
            wq_sb = constp.tile([128, 2, 768], BF16)
            nc.sync.dma_start(out=wq_sb[:], in_=wqkvT[:])
            wp_sb = constp.tile([128, 2, 256], BF16)
            nc.sync.dma_start(out=wp_sb[:], in_=wprojT[:])
            mask_sb = constp.tile([128, 128], BF16)
            nc.sync.dma_start(out=mask_sb[:], in_=maskc[:])
            ones_sb = constp.tile([128, 64], BF16)
            nc.sync.dma_start(out=ones_sb[:], in_=onesc[:])

            def _all():
                for b_ in range(img):
                    for (s0, ns) in ROUNDS:
                        _round(nc, sb, ps, x, y, b_, s0, ns,
                               wq_sb, wp_sb, mask_sb, ones_sb)

            if reps == 1:
                _all()
            else:
                with tc.For_i(0, reps, 1):
                    _all()
    return nc


def _round(nc, sb, ps, x, y, img, s0, ns, wq_sb, wp_sb, mask_sb, ones_sb):
    nt = ns * 288                  # tokens this round
    t0 = s0 * 288
    tiles = _tiles_for(ns)         # window counts per tile
    T = len(tiles)
    # token offsets per tile
    offs = np.cumsum([0] + [9 * w for w in tiles]).tolist()

    # chunks of <=4 tiles for qk / denom / ao (504-token granularity)
    chunks = []
    for c0 in range(0, T, 4):
        tl = list(range(c0, min(c0 + 4, T)))
        chunks.append((offs[tl[0]], tl))

    # ---- load xT channel-major via xbar transpose DMA (one per kc half) ----
    xT = sb.tile([128, 2, 2048], BF16, tag="xT", bufs=2)
    for cc in range(2):
        nc.sync.dma_start(out=xT[:, cc, 0:nt], in_=x[img, cc, t0:t0 + nt, :],
                          transpose=True)
    pad = min(nt + 128, 2048)
    nc.vector.memset(xT[:, :, nt:pad], 0.0)

    # ---- q^T, k^T channel-major; chunk mc holds heads (2mc, 2mc+1) ----
    qT = sb.tile([128, 2, 2048], BF16, tag="qT", bufs=2)
    kT = sb.tile([128, 2, 2048], BF16, tag="kT", bufs=2)
    nc.vector.memset(kT[:, :, nt:pad], 0.0)
    def _evac(eng, out, in_):
        if eng is nc.scalar:
            eng.copy(out=out, in_=in_)
        else:
            eng.tensor_copy(out=out, in_=in_)

    for t_, base, dst, eng in ((0, 0, qT, nc.vector), (1, 256, kT, nc.vector)):
        for mc in range(2):
            for (f0, tl) in chunks:
                nc_ = offs[tl[-1] + 1] - f0
                qp = ps.tile([128, 512], F32, tag="qk", bufs=2)
                for kc in range(2):
                    nc.tensor.matmul(
                        out=qp[:, 0:nc_],
                        lhsT=wq_sb[:, kc, base + 128 * mc: base + 128 * mc + 128],
                        rhs=xT[:, kc, f0:f0 + nc_],
                        start=(kc == 0),
                        stop=(kc == 1),
                    )
                _evac(eng, dst[:, mc, f0:f0 + nc_], qp[:, 0:nc_])

    # ---- v token-major per tile: [kb, 256] ----
    v_sb = sb.tile([126, 16, 256], BF16, tag="vs", bufs=2)
    for tp_ in range(0, T, 2):
        pair = [t for t in (tp_, tp_ + 1) if t < T]
        vp = ps.tile([128, 2, 256], F32, tag="sm", bufs=2)
        for j, t_ in enumerate(pair):
            f0 = offs[t_]
            for kc in range(2):
                nc.tensor.matmul(
                    out=vp[:, j, 0:256],
                    lhsT=xT[:, kc, f0:f0 + 128],
                    rhs=wq_sb[:, kc, 512:768],
                    start=(kc == 0),
                    stop=(kc == 1),
                )
        if len(pair) == 2 and tiles[pair[1]] == 14:
            nc.scalar.copy(out=v_sb[0:126, tp_:tp_ + 2, :], in_=vp[0:126, :, :])
        else:
            for j, t_ in enumerate(pair):
                kb = 9 * tiles[t_]
                nc.scalar.copy(out=v_sb[0:kb, t_, :], in_=vp[0:kb, j, :])

    # ---- QK^T logits per tile; exp -> expm [k, tile, (hh,mc), q] ----
    # row-group hh writes its own PSUM bank (free-offset 512*hh): mixing row
    # groups within one bank is an unrecoverable HW fault.
    expm = sb.tile([126, 16, 4, 126], BF16, tag="expm", bufs=2)
    if tiles[-1] != 14:
        nc.vector.memset(expm[:, T - 1, :, :], 0.0)
    for t_ in range(T):
        kb = 9 * tiles[t_]
        f0 = offs[t_]
        aL = ps.tile([128, 2, 512], F32, tag="att", bufs=2)
        for mc in range(2):
            for hh in range(2):
                p0 = 64 * hh
                nc.tensor.matmul(
                    out=aL[:, hh, 126 * mc: 126 * mc + kb],
                    lhsT=kT[p0:p0 + 64, mc, f0:f0 + 128],
                    rhs=qT[p0:p0 + 64, mc, f0:f0 + kb],
                    start=True,
                    stop=True,
                )
        ein = bass.AP(tensor=aL.tensor, offset=aL.offset,
                      ap=[[aL.ap[0][0], kb], [512, 2], [126, 2], [1, kb]])
        nc.scalar.activation(
            out=expm[0:kb, t_, 0:4, 0:kb], in_=ein,
            func=mybir.ActivationFunctionType.Exp, scale=SCALE)

    # ---- mask: expm *= blockdiag(9), one op over all tiles ----
    m = mask_sb[0:126, 0:126]
    Th_ = T // 2
    for lo, hi, eng in ((0, Th_, nc.vector), (Th_, T, nc.vector)):
        mb = bass.AP(tensor=m.tensor, offset=m.offset,
                     ap=[m.ap[0], [0, hi - lo], [0, 4], m.ap[1]])
        eng.tensor_mul(
            out=expm[:, lo:hi, :, :], in0=expm[:, lo:hi, :, :], in1=mb)

    # ---- denominators broadcast over 64-row groups via ones-matmul ----
    rbc = sb.tile([128, 2, 2048], F32, tag="rbc", bufs=2)
    for Th in range(2):
        dps = []
        for (f0, tl) in chunks:
            dp = ps.tile([128, 512], F32, tag="sm", bufs=2)
            for hh in range(2):
                h = 2 * Th + hh
                hc = 2 * (h % 2) + h // 2
                # per-tile rhs extents (tail tile is shorter)
                full = [t for t in tl if tiles[t] == 14]
                if full:
                    e0 = expm[0:126, full[0], hc, 0:126]
                    rhs = bass.AP(tensor=e0.tensor, offset=e0.offset,
                                  ap=[e0.ap[0], [4 * 126, len(full)],
                                      [1, 126]])
                    nc.tensor.matmul(
                        out=dp[64 * hh:64 * hh + 64,
                               0:126 * len(full)],
                        lhsT=ones_sb[0:126, 0:64],
                        rhs=rhs,
                        start=True, stop=True,
                        tile_position=(0, 64 * hh),
                    )
                for t in tl:
                    if tiles[t] == 14:
                        continue
                    kb = 9 * tiles[t]
                    nc.tensor.matmul(
                        out=dp[64 * hh:64 * hh + 64,
                               offs[t] - f0: offs[t] - f0 + kb],
                        lhsT=ones_sb[0:126, 0:64],
                        rhs=expm[0:126, t, hc, 0:kb],
                        start=True, stop=True,
                        tile_position=(0, 64 * hh),
                    )
            dps.append((f0, tl, dp))
        for (f0, tl, dp) in dps:
            nc_ = offs[tl[-1] + 1] - f0
            nc.vector.reciprocal_approx_fast(
                out=rbc[:, Th, f0:f0 + nc_], in_=dp[:, 0:nc_])

    # ---- AV: unnormalized channel-major ao; normalize during evac ----
    ao = sb.tile([128, 2, 2048], BF16, tag="ao", bufs=2)
    for Th in range(2):
        for (f0, tl) in chunks:
            nc_ = offs[tl[-1] + 1] - f0
            ap_ = ps.tile([128, 512], F32, tag="sm", bufs=2)
            for t in tl:
                kb = 9 * tiles[t]
                for hh in range(2):
                    h = 2 * Th + hh
                    hc = 2 * (h % 2) + h // 2
                    nc.tensor.matmul(
                        out=ap_[64 * hh:64 * hh + 64,
                                offs[t] - f0: offs[t] - f0 + kb],
                        lhsT=v_sb[0:kb, t, 64 * h: 64 * h + 64],
                        rhs=expm[0:kb, t, hc, 0:kb],
                        start=True, stop=True,
                        tile_position=(0, 64 * hh),
                    )
            nc.vector.tensor_mul(out=ao[:, Th, f0:f0 + nc_],
                                 in0=ap_[:, 0:nc_],
                                 in1=rbc[:, Th, f0:f0 + nc_])

    # ---- proj per 96-token chunk + evac to strip-major out_sb ----
    out_sb = sb.tile([96, 21, 256], BF16, tag="outs", bufs=2)
    ng = nt // 96
    for gp in range(0, ng, 2):
        pair = [g for g in (gp, gp + 1) if g < ng]
        op = ps.tile([128, 2, 256], F32, tag="sm", bufs=2)
        for j, g in enumerate(pair):
            for Th in range(2):
                nc.tensor.matmul(
                    out=op[0:96, j, 0:256],
                    lhsT=ao[:, Th, 96 * g: 96 * g + 96],
                    rhs=wp_sb[:, Th, :],
                    start=(Th == 0),
                    stop=(Th == 1),
                )
        nc.vector.tensor_copy(out=out_sb[0:96, gp:gp + len(pair), :],
                              in_=op[0:96, 0:len(pair), :])

    # ---- one output DMA for the whole round ----
    yout = bass.AP(tensor=y.tensor, offset=(img * NT + t0) * C,
                   ap=[[C, 96], [96 * C, 3 * ns], [1, C]])
    nc.sync.dma_start(out=yout, in_=out_sb[0:96, 0:3 * ns, 0:256])


def _make_consts():
    bf16 = ml_dtypes.bfloat16
    mask = np.zeros((128, 128), np.float32)
    for p in range(126):
        for q in range(126):
            if p // 9 == q // 9:
                mask[p, q] = 1.0
    return {
        "maskc": mask.astype(bf16),
        "onesc": np.ones((128, 64), np.float32).astype(bf16),
    }


_NC_CACHE = {}


def _get_nc():
    if "nc" not in _NC_CACHE:
        nc = bacc.Bacc("TRN2", target_bir_lowering=False, debug=False,
                       num_devices=NCORES)
        _build(nc)
        nc.compile()
        _NC_CACHE["nc"] = nc
    return _NC_CACHE["nc"]


def _perm_x(x):
    """[B, 9216, 256] f32 raster -> [B, 2, 9216, 128] bf16 window-contiguous
    (col-major within each 3-row strip), split by channel half."""
    bf16 = ml_dtypes.bfloat16
    x = np.asarray(x, np.float32).reshape(B, NSTRIP, 3, GRID, C)
    x = x.transpose(0, 1, 3, 2, 4).reshape(B, NT, C)       # col-major tokens
    x = x.reshape(B, NT, 2, 128).transpose(0, 2, 1, 3)     # ch-half major
    return np.ascontiguousarray(x).astype(bf16)


def _unperm_y(y):
    """[img, 9216, 256] bf16 col-major tokens -> [img, 9216, 256] f32 raster."""
    y = np.asarray(y, np.float32).reshape(-1, NSTRIP, GRID, 3, C)
    y = y.transpose(0, 1, 3, 2, 4).reshape(-1, NT, C)
    return y


def _in_maps(x, Wqkv, Wproj):
    bf16 = ml_dtypes.bfloat16
    consts = _make_consts()
    consts["wqkvT"] = np.ascontiguousarray(
        np.asarray(Wqkv, np.float32).T.reshape(2, 128, 768).transpose(1, 0, 2)
    ).astype(bf16)
    consts["wprojT"] = np.ascontiguousarray(
        np.asarray(Wproj, np.float32).T.reshape(2, 128, 256).transpose(1, 0, 2)
    ).astype(bf16)
    xp = _perm_x(x)
    return [{"x": xp[IMG * c: IMG * c + IMG], **consts} for c in range(NCORES)]


def kernel(x, Wqkv, Wproj, H, W):
    assert int(H) == GRID and int(W) == GRID
    nc = _get_nc()
    res = run_bass_kernel_spmd(nc, _in_maps(x, Wqkv, Wproj), list(range(NCORES)))
    out = np.concatenate([_unperm_y(res.results[c]["y"]) for c in range(NCORES)],
                         axis=0)
    return np.ascontiguousarray(out.reshape(B, NT, C)).astype(np.float32)
